# revision 3
# baseline (speedup 1.0000x reference)
"""PointsToVolumes (trilinear point splatting) on 8 TRN2 NeuronCores.

Full inputs -> full output. Sharding: core (b, q) owns output y-rows
[64q, 64q+64) of batch b, i.e. vol[b, :, :, 64q:64q+64, :].

Algorithm per core: points are grouped on host by (z-block, y-cell) into
128-point tiles. For each output row block (zb, Y) = [128 rows = (c, z_lo),
256 cols = x] f32 in PSUM, each contributing point tile adds
lhsT.T @ rhs where
  lhsT[k, c*64+zl] = amp_c[k] * wy_dy[k] * tent(z_lo[k])   (bf16, built on chip)
  rhs [k, x]       = tent(x[k])                            (bf16, built on chip)
tent(v)[j] = relu(1 - |j - v|) reproduces the trilinear weights exactly and
drops out-of-grid corners automatically. PSUM accumulates in f32; blocks are
evicted through SBUF to HBM once complete. No collectives are needed.
"""

import os
import sys
import types

import numpy as np

import concourse.bass as bass
import concourse.mybir as mybir
import concourse.tile as tile

# ---------------------------------------------------------------------------
# Container workarounds (this neuronxcc allows at most 1 sync wait per
# instruction and cannot compile Drain): split waits onto NOPs, skip the
# TileContext tail drain, and register the NTFF profiling hook.
# ---------------------------------------------------------------------------
if "antenv.axon_hooks" not in sys.modules:
    try:
        from trn_agent_boot.trn_boot import _ntff_profile_via_ctypes

        _mod = types.ModuleType("antenv.axon_hooks")
        _hook = _ntff_profile_via_ctypes("/opt/axon/libaxon_pjrt.so")
        _mod.get_axon_ntff_profile_hook = lambda: _hook
        sys.modules["antenv.axon_hooks"] = _mod
    except Exception:
        pass

import concourse.bass_utils as bu  # noqa: E402

bu.upload_artifacts = lambda tmpdir: "local://skipped"


def _nodrain(self, tick_clock, wait_clock):
    self.nc.all_engine_barrier()
    assert self.sems is not None
    popped = self.nc._tile_sem_poison_stack.pop()
    assert popped is self._sem_poison
    self.nc.clear_and_free_semaphores(list(self.sems.allocated().values()))
    self.nc.all_engine_barrier()


tile.TileContext._drain_and_barrier = _nodrain

_MAX_WAITS = 1
_nop_id = [0]


def _split_excess_waits(nc, max_waits=_MAX_WAITS):
    for f in nc.m.functions:
        for bb in f.blocks:
            ins = bb.instructions
            i = 0
            while i < len(ins):
                inst = ins[i]
                si = inst.sync_info
                if si is not None and si.on_wait and len(si.on_wait) > max_waits:
                    waits = list(si.on_wait)
                    excess, keep = waits[:-max_waits], waits[-max_waits:]
                    inst.sync_info = mybir.SyncInfo(
                        on_wait=keep, on_update=list(si.on_update)
                    )
                    while excess:
                        chunk, excess = excess[:max_waits], excess[max_waits:]
                        _nop_id[0] += 1
                        nop = mybir.InstNoOp(
                            name=f"waitnop-{_nop_id[0]}", ins=[], outs=[]
                        )
                        nop.engine = inst.engine
                        nop.sync_info = mybir.SyncInfo(on_wait=chunk, on_update=[])
                        ins.insert(i, nop)
                        i += 1
                i += 1


# ---------------------------------------------------------------------------
# Problem constants (hardcoded per the task contract).
# ---------------------------------------------------------------------------
G = 256          # grid side
NB = 2           # batches
NCH = 2          # amplitude channels
N = 100000       # points per batch
NQ = 4           # y-quarters (cores = NB * NQ = 8)
QH = G // NQ     # 64 y-rows per core
NZB = 4          # z-blocks
ZBH = G // NZB   # 64 z-planes per block
P = 128
dt = mybir.dt

_AP = mybir.AluOpType
_AF = mybir.ActivationFunctionType


# ---------------------------------------------------------------------------
# Host-side prep: group points into (z-block, y-cell) tiles per core with a
# structure (tile count per slot) shared by all cores so one SPMD program fits.
# ---------------------------------------------------------------------------
def _host_prep(positions, amplitudes):
    slots = [(zb, ycl) for zb in range(NZB) for ycl in range(-1, QH)]
    n_slots = len(slots)
    per_core = []  # per core: list over slots of dict arrays
    for b in range(NB):
        p = (positions[b].astype(np.float64) + 0.5) * G
        px, py, pz = (
            p[:, 0].astype(np.float32),
            p[:, 1].astype(np.float32),
            p[:, 2].astype(np.float32),
        )
        amp = amplitudes[b]
        y0 = np.floor(py).astype(np.int64)
        z0 = np.floor(pz).astype(np.int64)
        zb0 = z0 // ZBH
        strad_mask = (z0 % ZBH == ZBH - 1) & (z0 + 1 < G)
        for q in range(NQ):
            ylo, yhi = QH * q - 1, QH * q + QH - 1
            sel = (y0 >= ylo) & (y0 <= yhi)
            idx = np.nonzero(sel)[0]
            sid = idx[strad_mask[idx]]
            ent_pt = np.concatenate([idx, sid])
            ent_zb = np.concatenate([zb0[idx], zb0[sid] + 1])
            ent_yc = y0[ent_pt] - QH * q  # local cell in [-1, QH)
            key = ent_zb * (QH + 1) + (ent_yc + 1)
            order = np.argsort(key, kind="stable")
            ent_pt, key = ent_pt[order], key[order]
            counts = np.bincount(key, minlength=n_slots)
            starts = np.concatenate([[0], np.cumsum(counts)])
            core = {
                "pt": ent_pt,
                "counts": counts,
                "starts": starts,
                "px": px,
                "py": py,
                "pz": pz,
                "amp": amp,
                "q": q,
            }
            per_core.append(core)

    counts_all = np.stack([c["counts"] for c in per_core])  # [8, n_slots]
    ntiles = np.maximum((counts_all.max(0) + P - 1) // P, 1).astype(np.int64)
    T = int(ntiles.sum())

    in_maps = []
    for core in per_core:
        NPX = np.zeros((P, T), np.float32)
        NPZL = np.zeros((P, T), np.float32)
        FY = np.zeros((P, T), np.float32)
        A0 = np.zeros((P, T), np.float32)
        A1 = np.zeros((P, T), np.float32)
        tcol = 0
        for si, (zb, ycl) in enumerate(slots):
            nt = int(ntiles[si])
            s, e = core["starts"][si], core["starts"][si + 1]
            pts = core["pt"][s:e]
            n = len(pts)
            cap = nt * P
            assert n <= cap
            col = np.zeros((cap,), np.float32)

            def put(dst, vals, fill=0.0):
                col = np.full((cap,), fill, np.float32)
                col[:n] = vals
                dst[:, tcol:tcol + nt] = col.reshape(nt, P).T

            put(NPX, -core["px"][pts], fill=-1e4)
            put(NPZL, ZBH * zb - core["pz"][pts], fill=-1e4)
            put(FY, core["py"][pts] - (QH * core["q"] + ycl), fill=0.0)
            put(A0, core["amp"][0, pts], fill=0.0)
            put(A1, core["amp"][1, pts], fill=0.0)
            tcol += nt
        in_maps.append({"NPX": NPX, "NPZL": NPZL, "FY": FY, "A0": A0, "A1": A1})
    return slots, ntiles, T, in_maps


# ---------------------------------------------------------------------------
# Device program
# ---------------------------------------------------------------------------
def _build_program(slots, ntiles, T):
    nc = bass.Bass()
    NPX = nc.declare_dram_parameter("NPX", [P, T], dt.float32, isOutput=False)
    NPZL = nc.declare_dram_parameter("NPZL", [P, T], dt.float32, isOutput=False)
    FY = nc.declare_dram_parameter("FY", [P, T], dt.float32, isOutput=False)
    A0 = nc.declare_dram_parameter("A0", [P, T], dt.float32, isOutput=False)
    A1 = nc.declare_dram_parameter("A1", [P, T], dt.float32, isOutput=False)
    OUT = nc.declare_dram_parameter("OUT", [NCH, G, QH, G], dt.float32,
                                    isOutput=True)

    with tile.TileContext(nc) as tc:
        with (
            tc.tile_pool(name="const", bufs=1) as cpool,
            tc.tile_pool(name="batch", bufs=1) as bpool,
            tc.tile_pool(name="work", bufs=4) as wpool,
            tc.tile_pool(name="stage", bufs=4) as spool,
            tc.tile_pool(name="psum", bufs=3, space="PSUM") as ppool,
        ):
            # constants
            iota64 = cpool.tile([P, ZBH], dt.bfloat16)
            iota256 = cpool.tile([P, G], dt.bfloat16)
            nc.gpsimd.iota(iota64[:], pattern=[[1, ZBH]], base=0,
                           channel_multiplier=0,
                           allow_small_or_imprecise_dtypes=True)
            nc.gpsimd.iota(iota256[:], pattern=[[1, G]], base=0,
                           channel_multiplier=0,
                           allow_small_or_imprecise_dtypes=True)

            # load point arrays
            npx_t = bpool.tile([P, T], dt.float32)
            npzl_t = bpool.tile([P, T], dt.float32)
            fy_t = bpool.tile([P, T], dt.float32)
            a0_t = bpool.tile([P, T], dt.float32)
            a1_t = bpool.tile([P, T], dt.float32)
            nc.sync.dma_start(out=npx_t[:], in_=NPX[:])
            nc.sync.dma_start(out=npzl_t[:], in_=NPZL[:])
            nc.sync.dma_start(out=fy_t[:], in_=FY[:])
            nc.sync.dma_start(out=a0_t[:], in_=A0[:])
            nc.sync.dma_start(out=a1_t[:], in_=A1[:])

            # batched point math: s{c}{dy} = amp_c * wy_dy
            w0_t = bpool.tile([P, T], dt.float32)
            s00_t = bpool.tile([P, T], dt.float32)
            s01_t = bpool.tile([P, T], dt.float32)
            s10_t = bpool.tile([P, T], dt.float32)
            s11_t = bpool.tile([P, T], dt.float32)
            nc.vector.tensor_scalar(out=w0_t[:], in0=fy_t[:], scalar1=-1.0,
                                    scalar2=1.0, op0=_AP.mult, op1=_AP.add)
            nc.vector.tensor_tensor(out=s00_t[:], in0=a0_t[:], in1=w0_t[:],
                                    op=_AP.mult)
            nc.vector.tensor_tensor(out=s01_t[:], in0=a0_t[:], in1=fy_t[:],
                                    op=_AP.mult)
            nc.vector.tensor_tensor(out=s10_t[:], in0=a1_t[:], in1=w0_t[:],
                                    op=_AP.mult)
            nc.vector.tensor_tensor(out=s11_t[:], in0=a1_t[:], in1=fy_t[:],
                                    op=_AP.mult)

            # main loop
            tcol = 0
            blocks = {}      # yc_row -> (psum_tile, n_contrib_done)
            started = set()  # rows with first matmul emitted
            # number of matmuls each row block will receive, for stop flags
            for zbi in range(NZB):
                slot_list = [(si, s) for si, s in enumerate(slots) if s[0] == zbi]
                contrib = {}
                for si, (zb, ycl) in slot_list:
                    nt = int(ntiles[si])
                    if ycl >= 0:
                        contrib[ycl] = contrib.get(ycl, 0) + nt
                    if ycl + 1 < QH:
                        contrib[ycl + 1] = contrib.get(ycl + 1, 0) + nt
                done = {}
                for si, (zb, ycl) in slot_list:
                    nt = int(ntiles[si])
                    for j in range(nt):
                        t = tcol + j
                        # z tent [P, 64]
                        az = wpool.tile([P, ZBH], dt.bfloat16, tag="az")
                        tz = wpool.tile([P, ZBH], dt.bfloat16, tag="tz")
                        nc.scalar.activation(az[:], iota64[:], _AF.Abs,
                                             bias=npzl_t[:, t, None])
                        nc.scalar.activation(tz[:], az[:], _AF.Relu,
                                             bias=1.0, scale=-1.0)
                        # x tent [P, 256]
                        ax = wpool.tile([P, G], dt.bfloat16, tag="ax")
                        tx = wpool.tile([P, G], dt.bfloat16, tag="tx")
                        nc.scalar.activation(ax[:], iota256[:], _AF.Abs,
                                             bias=npx_t[:, t, None])
                        nc.scalar.activation(tx[:], ax[:], _AF.Relu,
                                             bias=1.0, scale=-1.0)
                        for dy in (0, 1):
                            Y = ycl + dy
                            if Y < 0 or Y >= QH:
                                continue
                            sA = (s00_t, s01_t)[dy]
                            sB = (s10_t, s11_t)[dy]
                            lh = wpool.tile([P, P], dt.bfloat16, tag="lh")
                            nc.vector.tensor_scalar(
                                out=lh[:, 0:ZBH], in0=tz[:],
                                scalar1=sA[:, t, None], scalar2=None,
                                op0=_AP.mult)
                            nc.vector.tensor_scalar(
                                out=lh[:, ZBH:2 * ZBH], in0=tz[:],
                                scalar1=sB[:, t, None], scalar2=None,
                                op0=_AP.mult)
                            if Y not in blocks:
                                blocks[Y] = ppool.tile([P, G], dt.float32,
                                                       tag="blk", name=f"blk{zbi}_{Y}")
                            ps = blocks[Y]
                            first = Y not in started
                            started.add(Y)
                            d = done.get(Y, 0) + 1
                            done[Y] = d
                            nc.tensor.matmul(out=ps[:], lhsT=lh[:], rhs=tx[:],
                                             start=first,
                                             stop=(d == contrib[Y]))
                    tcol += nt
                    # slot finished; row ycl (if valid) is complete
                    if ycl >= 0:
                        ps = blocks.pop(ycl)
                        started.discard(ycl)
                        st = spool.tile([P, G], dt.float32, tag="st")
                        nc.vector.tensor_copy(out=st[:], in_=ps[:])
                        nc.sync.dma_start(
                            out=OUT[:, zbi * ZBH:(zbi + 1) * ZBH, ycl, :],
                            in_=st[:],
                        )
                assert not blocks, (zbi, blocks.keys())
    return nc


_PROGRAM_CACHE = {}


def kernel(positions, amplitudes, trace=False, tmpdir=None):
    positions = np.asarray(positions)
    amplitudes = np.asarray(amplitudes)
    slots, ntiles, T, in_maps = _host_prep(positions, amplitudes)

    key = (T, tuple(int(x) for x in ntiles))
    if key not in _PROGRAM_CACHE:
        nc = _build_program(slots, ntiles, T)
        _split_excess_waits(nc)
        _PROGRAM_CACHE[key] = nc
    nc = _PROGRAM_CACHE[key]

    core_ids = list(range(NB * NQ))
    res = bu.run_bass_kernel_spmd(nc, in_maps, core_ids, trace=trace,
                                  tmpdir=tmpdir)

    out = np.zeros((NB, NCH, G, G, G), np.float32)
    for cid in core_ids:
        b, q = divmod(cid, NQ)
        out[b, :, :, QH * q:QH * q + QH, :] = res.results[cid]["OUT"]
    if trace:
        kernel.last_exec_ns = res.exec_time_ns
    return out


kernel.last_exec_ns = None
